# revision 1
# baseline (speedup 1.0000x reference)
"""MultiHeadDualAttention Trainium2 kernel.

Sharding: 8 heads -> 8 cores (tensor parallel over heads). Each core gets the
full k1/v1/k2/v2 (pre-transposed on host to [256, 4096] so the contraction dim
lands on SBUF partitions) plus its head's slices of the wk/wv/wo weights.

Math per head (verified exact vs reference in fp64):
  o2 = rowsoftmax(S_true) @ v2p_full ; o1 = colsoftmax(S_true)^T @ v1p_full
  - v-bias cancels through softmax row-sums == 1, re-added on host via
    (bv @ wo + bo) constants.
  - k-bias: rowsoftmax(S_true) == rowsoftmax(k1p_FULL @ k2p_NOB^T) and
    colsoftmax(S_true) == colsoftmax(k1p_NOB @ k2p_FULL^T), so each direction
    uses one biased and one unbiased projection and no rank-1 corrections.
  - exp without max-subtraction: |SCALE*S| < ~2.5, safe in fp32.
  - softmax denominators exported unnormalized (den1/den2); host divides.

Perf structure:
  - wk weights are shipped column-duplicated [256, 128] so the k projections
    land duplicated on both partition halves; the K=64 score matmuls then run
    2x row-packed (64x128 PE tiles T0/T8 via base_partition 0/64).
  - E is built in [128, 16, 1024] bf16 sub-blocks (exp at FD=1024 amortizes
    the ACT instruction overhead), double-buffered against the PV matmuls.
  - PV uses a ones-augmented V (M=65) accumulating [65, 512] in PSUM over all
    32 partition-tiles; row 64 is the softmax denominator.
Output projection wo is applied on-device per head ([64,256] slice); host sums
the 8 partial [256, 4096] results (the "all-reduce" of the row-sharded wo).
"""

import sys

sys.path.insert(0, "/opt/trn_rl_repo")

import numpy as np

N = 4096
C = 256
AD = 512
H = 8
D = 64
SCALE = float(D) ** -0.5
NCORES = 8
NBLK = 1024         # free-dim block width for E
NCHUNK = N // NBLK  # 4 blocks
MT = N // 128       # 32 partition-tiles of E per block
SUB = 4             # m-tiles per E sub-block

_cache: dict = {}


def _build_module():
    import concourse.bacc as bacc
    import concourse.mybir as mybir
    import concourse.tile as tile

    f32 = mybir.dt.float32
    bf16 = mybir.dt.bfloat16
    Exp = mybir.ActivationFunctionType.Exp

    nc = bacc.Bacc("TRN2", target_bir_lowering=False, debug=False)

    def din(name, shape, dt=bf16):
        return nc.dram_tensor(name, shape, dt, kind="ExternalInput").ap()

    def dout(name, shape):
        return nc.dram_tensor(name, shape, f32, kind="ExternalOutput").ap()

    k1T = din("k1T", [C, N])
    v1T = din("v1T", [C, N])
    k2T = din("k2T", [C, N])
    v2T = din("v2T", [C, N])
    wk1 = din("wk1", [C, 128])   # column-duplicated [wk|wk]
    wk2 = din("wk2", [C, 128])
    wv1 = din("wv1", [C, D])
    wv2 = din("wv2", [C, D])
    bk1 = din("bk1", [128, 1], f32)  # row-duplicated
    bk2 = din("bk2", [128, 1], f32)
    wo1 = din("wo1", [D, C])
    wo2 = din("wo2", [D, C])

    o1pT = dout("o1pT", [C, N])
    o2pT = dout("o2pT", [C, N])
    den1 = dout("den1", [1, N])
    den2 = dout("den2", [1, N])

    with tile.TileContext(nc) as tc:
        with (
            tc.tile_pool(name="const", bufs=1) as constp,
            tc.tile_pool(name="raw", bufs=8) as rawp,
            tc.tile_pool(name="eblk", bufs=8) as ep,
            tc.tile_pool(name="outp", bufs=3) as outp,
            tc.tile_pool(name="spsum", bufs=2, space="PSUM") as spsum,
            tc.tile_pool(name="opsum", bufs=4, space="PSUM") as opsum,
        ):
            # ---- PE warm-up: ~7us of dummy matmuls so the HAM clock-gate
            # reaches K=8/8 (2.4 GHz) before the real work arrives ----
            warm = constp.tile([128, 512], bf16, tag="warm")
            nc.gpsimd.memset(warm[:], 0.0)
            wps = opsum.tile([128, 512], f32, tag="op", name="warm_ps")
            for _ in range(16):
                nc.tensor.matmul(wps[:], warm[:, 0:128], warm[:], start=True, stop=True)

            # ---- load weights ----
            w_sb = {}
            for name, drt, w in (("wk1", wk1, 128), ("wk2", wk2, 128),
                                 ("wv1", wv1, D), ("wv2", wv2, D)):
                t = constp.tile([128, 2, w], bf16, tag=name)
                for ct in range(2):
                    nc.sync.dma_start(out=t[:, ct, :], in_=drt[ct * 128:(ct + 1) * 128, :])
                w_sb[name] = t
            bk1_sb = constp.tile([128, 1], f32, tag="bk1")
            nc.sync.dma_start(out=bk1_sb[:], in_=bk1[:])
            bk2_sb = constp.tile([128, 1], f32, tag="bk2")
            nc.sync.dma_start(out=bk2_sb[:], in_=bk2[:])
            wo1_sb = constp.tile([D, C], bf16, tag="wo1")
            nc.sync.dma_start(out=wo1_sb[:], in_=wo1[:])
            wo2_sb = constp.tile([D, C], bf16, tag="wo2")
            nc.sync.dma_start(out=wo2_sb[:], in_=wo2[:])

            # ---- k projections: [128, 4096] bf16, data duplicated on both
            # partition halves (weights are column-duplicated) ----
            def k_proj(rawT, w, b_sb, tagbase):
                # chunked tiles so score matmuls can start before the whole
                # projection finishes (Tile deps are per-tile)
                nob, full = [], []
                for j in range(8):
                    raw = rawp.tile([128, 2, 512], bf16, tag="raw")
                    for ct in range(2):
                        nc.sync.dma_start(
                            out=raw[:, ct, :],
                            in_=rawT[ct * 128:(ct + 1) * 128, j * 512:(j + 1) * 512],
                        )
                    ps = opsum.tile([128, 512], f32, tag="op")
                    for ct in range(2):
                        nc.tensor.matmul(
                            ps[:], w[:, ct, :], raw[:, ct, :],
                            start=(ct == 0), stop=(ct == 1),
                        )
                    nobj = constp.tile([128, 512], bf16, tag=f"{tagbase}_nob{j}")
                    fullj = constp.tile([128, 512], bf16, tag=f"{tagbase}_full{j}")
                    nc.vector.tensor_copy(nobj[:], ps[:])
                    nc.vector.tensor_scalar_add(fullj[:], nobj[:], b_sb[:])
                    nob.append(nobj)
                    full.append(fullj)
                return nob, full

            k1_nob, k1_full = k_proj(k1T, w_sb["wk1"], bk1_sb, "k1p")
            k2_nob, k2_full = k_proj(k2T, w_sb["wk2"], bk2_sb, "k2p")

            # ---- v projections: [128, 32, 65] bf16, ones in col 64 ----
            def v_proj(rawT, w, tagbase):
                vaug = constp.tile([128, MT, D + 1], bf16, tag=tagbase)
                nc.vector.memset(vaug[:, :, D:D + 1], 1.0)
                for j in range(8):
                    raw = rawp.tile([128, 2, 512], bf16, tag="raw")
                    for ct in range(2):
                        nc.sync.dma_start(
                            out=raw[:, ct, :],
                            in_=rawT[ct * 128:(ct + 1) * 128, j * 512:(j + 1) * 512],
                        )
                    for k in range(4):
                        nt = j * 4 + k
                        ps = opsum.tile([128, D], f32, tag="op")
                        for ct in range(2):
                            nc.tensor.matmul(
                                ps[:], raw[:, ct, k * 128:(k + 1) * 128],
                                w[:, ct, :],
                                start=(ct == 0), stop=(ct == 1),
                            )
                        nc.vector.tensor_copy(vaug[:, nt, :D], ps[:])
                return vaug


            # ---- one softmax direction ----
            def attention_pass(kP, kF, vaug, oT_tag, proj=None):
                """E[p, f] = exp(SCALE * kP[:,p]^T kF[:,f]); oT = [vaug|1]^T E.

                kP/kF are chunked partition-duplicated [128, 512] tiles; score
                matmuls run as 2x row-packed 64x128 PE tiles (T0/T8).
                """
                oT, dsb = [], []
                for j in range(NCHUNK):
                    po = [opsum.tile([D + 1, 512], f32, tag="op", name=f"po_{oT_tag}_{j}_{c}")
                          for c in range(2)]
                    for sub in range(MT // SUB):
                        eblk = ep.tile([128, SUB, NBLK], bf16, tag="eblk")
                        for pair in range(SUB // 2):
                            mtA = sub * SUB + 2 * pair
                            mtB = mtA + 1
                            psA = spsum.tile([128, NBLK], f32, tag="sp")
                            psB = spsum.tile([128, NBLK], f32, tag="sp")
                            for c in range(2):
                                for ps, mt, lo, hi in ((psA, mtA, 0, 64), (psB, mtB, 64, 128)):
                                    nc.tensor.matmul(
                                        ps[:, c * 512:(c + 1) * 512],
                                        kP[mt // 4][lo:hi, (mt % 4) * 128:(mt % 4 + 1) * 128],
                                        kF[2 * j + c][lo:hi, :],
                                        start=True, stop=True,
                                    )
                            nc.scalar.activation(eblk[:, 2 * pair, :], psA[:], Exp, scale=SCALE)
                            nc.scalar.activation(eblk[:, 2 * pair + 1, :], psB[:], Exp, scale=SCALE)
                        for c in range(2):
                            for mtl in range(SUB):
                                mt = sub * SUB + mtl
                                nc.tensor.matmul(
                                    po[c][:], vaug[:, mt, :],
                                    eblk[:, mtl, c * 512:(c + 1) * 512],
                                    start=(mt == 0), stop=(mt == MT - 1),
                                )
                    for c in range(2):
                        jc = 2 * j + c
                        oc = constp.tile([D, 512], bf16, tag=f"{oT_tag}_o{jc}")
                        dc = constp.tile([1, 512], f32, tag=f"{oT_tag}_d{jc}")
                        nc.vector.tensor_copy(oc[:], po[c][0:D, :])
                        nc.vector.tensor_copy(dc[:], po[c][D:D + 1, :])
                        oT.append(oc)
                        dsb.append(dc)
                        if proj is not None:
                            # stream this chunk's wo projection + DMA now so the
                            # kernel tail is only the last chunk's epilogue
                            wo_sb, outdr, dendr = proj
                            for ct in range(2):
                                pp = opsum.tile([128, 512], f32, tag="op")
                                nc.tensor.matmul(
                                    pp[:], wo_sb[:, ct * 128:(ct + 1) * 128],
                                    oc[:], start=True, stop=True,
                                )
                                ot = outp.tile([128, 512], f32, tag="out")
                                nc.vector.tensor_copy(ot[:], pp[:])
                                nc.sync.dma_start(
                                    out=outdr[ct * 128:(ct + 1) * 128,
                                              jc * 512:(jc + 1) * 512],
                                    in_=ot[:],
                                )
                            nc.sync.dma_start(
                                out=dendr[0:1, jc * 512:(jc + 1) * 512], in_=dc[:])
                return oT, dsb

            # ---- output projections (per-head slice of wo) ----
            def out_proj(oT, dsb, wo_sb, outdr, dendr):
                for j in range(8):
                    for ct in range(2):
                        pp = opsum.tile([128, 512], f32, tag="op")
                        nc.tensor.matmul(
                            pp[:], wo_sb[:, ct * 128:(ct + 1) * 128],
                            oT[j][:],
                            start=True, stop=True,
                        )
                        ot = outp.tile([128, 512], f32, tag="out")
                        nc.vector.tensor_copy(ot[:], pp[:])
                        nc.sync.dma_start(
                            out=outdr[ct * 128:(ct + 1) * 128, j * 512:(j + 1) * 512],
                            in_=ot[:],
                        )
                    nc.sync.dma_start(out=dendr[0:1, j * 512:(j + 1) * 512], in_=dsb[j][:])

            v1_aug = v_proj(v1T, w_sb["wv1"], "v1aug")
            v2_aug = v_proj(v2T, w_sb["wv2"], "v2aug")

            # o2: E[m, n] from k2p_nob (partition) x k1p_full (free)
            o2T, d2sb = attention_pass(k2_nob, k1_full, v2_aug, "o2T")
            # o2's projection interleaves into the o1 pass (PE gap-filler)
            out_proj(o2T, d2sb, wo2_sb, o2pT, den2)
            # o1: E[n, m] from k1p_nob (partition) x k2p_full (free);
            # its wo projection streams per-block inside the pass
            attention_pass(k1_nob, k2_full, v1_aug, "o1T",
                           proj=(wo1_sb, o1pT, den1))

    nc.compile()
    return nc


def _get_nc():
    if "nc" not in _cache:
        _cache["nc"] = _build_module()
    return _cache["nc"]


def kernel(k1, v1, k2, v2,
           wk1_w, wk1_b, wv1_w, wv1_b,
           wk2_w, wk2_b, wv2_w, wv2_b,
           wo1_w, wo1_b, wo2_w, wo2_b):
    import ml_dtypes
    from concourse.bass_utils import run_bass_kernel_spmd

    nc = _get_nc()

    f = np.float32
    bf = ml_dtypes.bfloat16
    k1T = np.ascontiguousarray(np.asarray(k1, f).T).astype(bf)
    v1T = np.ascontiguousarray(np.asarray(v1, f).T).astype(bf)
    k2T = np.ascontiguousarray(np.asarray(k2, f).T).astype(bf)
    v2T = np.ascontiguousarray(np.asarray(v2, f).T).astype(bf)

    def dup2(a):  # [C, D] -> [C, 128] column-duplicated
        return np.ascontiguousarray(np.concatenate([a, a], axis=1))

    in_maps = []
    for h in range(NCORES):
        sl = slice(h * D, (h + 1) * D)
        in_maps.append({
            "k1T": k1T, "v1T": v1T, "k2T": k2T, "v2T": v2T,
            "wk1": dup2(np.asarray(wk1_w, f)[:, sl]).astype(bf),
            "wv1": np.ascontiguousarray(np.asarray(wv1_w, f)[:, sl]).astype(bf),
            "wk2": dup2(np.asarray(wk2_w, f)[:, sl]).astype(bf),
            "wv2": np.ascontiguousarray(np.asarray(wv2_w, f)[:, sl]).astype(bf),
            "bk1": np.ascontiguousarray(np.tile(np.asarray(wk1_b, f)[sl].reshape(D, 1), (2, 1))),
            "bk2": np.ascontiguousarray(np.tile(np.asarray(wk2_b, f)[sl].reshape(D, 1), (2, 1))),
            "wo1": np.ascontiguousarray(np.asarray(wo1_w, f)[sl, :]).astype(bf),
            "wo2": np.ascontiguousarray(np.asarray(wo2_w, f)[sl, :]).astype(bf),
        })

    res = run_bass_kernel_spmd(nc, in_maps, list(range(NCORES)))
    _cache["last_result"] = res

    out1 = np.zeros((N, C), np.float32)
    out2 = np.zeros((N, C), np.float32)
    for h in range(NCORES):
        rh = res.results[h]
        out1 += (rh["o1pT"] / rh["den1"]).T
        out2 += (rh["o2pT"] / rh["den2"]).T
    # v-bias and output bias constants (v-bias commutes through softmax).
    out1 += np.asarray(wv1_b, f) @ np.asarray(wo1_w, f) + np.asarray(wo1_b, f)
    out2 += np.asarray(wv2_b, f) @ np.asarray(wo2_w, f) + np.asarray(wo2_b, f)
    return out1, out2



# revision 2
# speedup vs baseline: 1.0021x; 1.0021x over previous
"""MultiHeadDualAttention Trainium2 kernel, v5 (fp8 DoubleRow + split exp).

Sharding: 8 heads -> 8 cores. Each core: full k1/k2/v1/v2 (host-transposed to
[256, 4096] bf16) + its head's wk/wv slices. Outputs per core: unnormalized
o1T/o2T [65, 4096] bf16 (row 64 = softmax denominator); host divides, applies
the wo projection (row-shard of wo = per-head slice, concat over heads), and
adds the v-bias/wo-bias constants (v-bias commutes through softmax).

Math per head: S[n, m] = k1F[n]·k2F[m] with k1F/k2F the *biased* projections;
rowsoftmax(S) / colsoftmax(S) are exactly the reference's two directions.
Each direction builds its E matrix in the contract-on-partitions layout:
  o2: E[m, n] = exp(SCALE*k2F[m]·k1F[n]), contract over m
  o1: E[n, m] = exp(SCALE*k1F[n]·k2F[m]), contract over n

Perf structure (fp8e4 hot loop):
  - k projections staged fp8 into a DoubleRow layout [32p, 2kt, 4096]
    (d = 2i+t), duplicated at all four partition quarters: the four score
    matmuls of a quad run on PE quarter-tiles (rows 0/32/64/96) as
    concurrent streams.
  - exp: split ACT (exact Exp -> fp8) / DVE (Schraudolph: rn(S*a+b) -> int8
    bitcast fp8e4), strictly alternating; fp8 quantization dominates the
    error of both paths. Measured end relerr ~1.3e-2 (gate 2e-2).
  - PV: fp8 DoubleRow, one matmul per m-tile pair ([128, 2, 512] E x
    [128, 2, 80] v-aug; row pitch 80 because dual-fp8 ldweights needs
    16-divisible strides), accumulating [80, 512] over 16 pairs; row 64
    (ones column) is the softmax denominator.
  - HAM (PE clock-gate at K=4/8 = 1.2 GHz unless kept busy): the PE must be
    the 100%-duty bottleneck engine or the gate re-throttles and never
    recovers (a dependency-stalled stream never reads "busy"). Hence: PE
    work per quad (~1.4us) deliberately exceeds the exp cadence (~1.17us),
    a dependency-free warm burst behind a scheduler fence flips the gate
    right before the attention, and extra burst matmuls are woven between
    the first quads to cover the pipeline-fill stalls.
  - wo projection + normalization on host.
"""

import sys

sys.path.insert(0, "/opt/trn_rl_repo")

import numpy as np

N = 4096
C = 256
D = 64
SCALE = float(D) ** -0.5
NCORES = 8
NCH = 8          # n-chunks of 512
PAIRS = 16       # m-tile pairs (32 m-tiles)
A_SCH = float(8.0 * np.log2(np.e) * SCALE)   # schraudolph multiplier
B_SCH = 55.8                                  # schraudolph magic bias

_cache: dict = {}


def _build_module():
    import concourse.bacc as bacc
    import concourse.mybir as mybir
    import concourse.tile as tile

    f32 = mybir.dt.float32
    bf16 = mybir.dt.bfloat16
    fp8 = mybir.dt.float8e4
    i8 = mybir.dt.int8
    Exp = mybir.ActivationFunctionType.Exp
    Ident = mybir.ActivationFunctionType.Identity
    DR = mybir.MatmulPerfMode.DoubleRow
    Alu = mybir.AluOpType

    nc = bacc.Bacc("TRN2", target_bir_lowering=False, debug=False)

    def din(name, shape, dt=bf16):
        return nc.dram_tensor(name, shape, dt, kind="ExternalInput").ap()

    def dout(name, shape, dt):
        return nc.dram_tensor(name, shape, dt, kind="ExternalOutput").ap()

    k1T = din("k1T", [C, N])
    v1T = din("v1T", [C, N])
    k2T = din("k2T", [C, N])
    v2T = din("v2T", [C, N])
    wk1 = din("wk1", [C, D])
    wk2 = din("wk2", [C, D])
    wv1 = din("wv1", [C, D])
    wv2 = din("wv2", [C, D])
    bk1 = din("bk1", [D, 1], f32)
    bk2 = din("bk2", [D, 1], f32)

    o1Td = dout("o1T", [D + 1, N], bf16)
    o2Td = dout("o2T", [D + 1, N], bf16)

    exp_idx = [0]

    with tile.TileContext(nc) as tc:
        with (
            tc.tile_pool(name="const", bufs=1) as constp,
            tc.tile_pool(name="stg", bufs=4) as stgp,
            tc.tile_pool(name="eblk", bufs=8) as ep,
            tc.tile_pool(name="outp", bufs=4) as outp,
            tc.tile_pool(name="spsum", bufs=3, space="PSUM") as spsum,
            tc.tile_pool(name="opsum", bufs=2, space="PSUM") as opsum,
        ):
            # ---- weights ----
            w_sb = {}
            for name, drt in (("wk1", wk1), ("wk2", wk2), ("wv1", wv1), ("wv2", wv2)):
                t = constp.tile([128, 2, D], bf16, tag=name, name=f"w_{name}")
                for ct in range(2):
                    nc.sync.dma_start(out=t[:, ct, :], in_=drt[ct * 128:(ct + 1) * 128, :])
                w_sb[name] = t
            bk1_sb = constp.tile([D, 1], f32, tag="bk1")
            nc.sync.dma_start(out=bk1_sb[:], in_=bk1[:])
            bk2_sb = constp.tile([D, 1], f32, tag="bk2")
            nc.sync.dma_start(out=bk2_sb[:], in_=bk2[:])

            warm = constp.tile([128, 512], bf16, tag="warm")
            nc.gpsimd.memset(warm[:], 0.0)

            # ---- k projections -> fp8 DoubleRow layout, quarter-duplicated
            # kf[q*32 + i, t, n] = k_proj[d = 2i + t, n] + bias, q in 0..4 ----
            def k_proj_unit(kf, raws, w, b_sb, tag, u):
                stg = stgp.tile([D, 2, 512], fp8, tag="kstg", name=f"kstg_{tag}{u}")
                for jj in range(2):
                    j = 2 * u + jj
                    raw = raws[j]
                    kps = opsum.tile([80, 512], f32, tag="po", name=f"kps_{tag}{j}")
                    for ct in range(2):
                        nc.tensor.matmul(
                            kps[0:D, :], w[:, ct, :], raw[:, ct, :],
                            start=(ct == 0), stop=(ct == 1),
                        )
                    nc.scalar.activation(stg[:, jj, :], kps[0:D, :], Ident, bias=b_sb[:])
                # staging DMAs on the ACT hwdge queue (the SP queue is busy
                # with raw loads; queue-order there would gate the attention)
                for q in range(4):
                    nc.scalar.dma_start(
                        out=kf[q * 32:(q + 1) * 32, :, (2 * u) * 512:(2 * u + 2) * 512],
                        in_=stg[:],
                    )

            # batched raw loads: the head is DMA issue-rate bound (~0.9us of
            # queue time per dma_start), so load 2 chunks per DMA and
            # interleave k1/k2/v2 so v2 lands early (v2aug gates the first
            # PV). The [256, 1024] DRAM block maps to [ct, p, j, n] SBUF
            # order via a rearranged AP (c = ct*128 + p).
            def load_raw(rawT, tg):
                t = constp.tile([128, 2, NCH, 512], bf16, tag=f"{tg}raw",
                                name=f"rawt_{tg}")
                return t

            rawt = {tg: load_raw(rawT, tg)
                    for tg, rawT in (("k1", k1T), ("k2", k2T), ("v2", v2T), ("v1", v1T))}
            for u in range(NCH // 2):
                for tg, rawT in (("k1", k1T), ("k2", k2T), ("v2", v2T)):
                    nc.sync.dma_start(
                        out=rawt[tg][:, :, 2 * u:2 * u + 2, :],
                        in_=rawT[:, (2 * u) * 512:(2 * u + 2) * 512].rearrange(
                            "(c p) (j n) -> p c j n", c=2, j=2),
                    )
            k1raws = [rawt["k1"][:, :, j, :] for j in range(NCH)]
            k2raws = [rawt["k2"][:, :, j, :] for j in range(NCH)]
            v2raws = [rawt["v2"][:, :, j, :] for j in range(NCH)]

            k1f = constp.tile([128, 2, N], fp8, tag="k1f", name="kf_k1f")
            k2f = constp.tile([128, 2, N], fp8, tag="k2f", name="kf_k2f")
            # interleave k1/k2 units so staging issue tracks cast completion
            for u in range(NCH // 2):
                k_proj_unit(k1f, k1raws, w_sb["wk1"], bk1_sb, "k1f", u)
                k_proj_unit(k2f, k2raws, w_sb["wk2"], bk2_sb, "k2f", u)

            # ---- v projections -> fp8 v-aug [128, pair, kt, 80] ----
            # (col 64 = ones for the softmax denominator, 65-79 zero pad)
            def v_proj_compute(raws, w, tag):
                vaug = constp.tile([128, 16, 2, 80], fp8, tag=tag, name=f"vaug_{tag}")
                nc.vector.memset(vaug[:, :, :, D:80], 0.0)
                nc.vector.memset(vaug[:, :, :, D:D + 1], 1.0)
                for half in range(2):
                    vps = spsum.tile([128, 2, 512], f32, tag="sAB", name=f"vps_{tag}{half}")
                    for jj in range(4):
                        raw = raws[half * 4 + jj]
                        for k in range(4):
                            nt_loc = jj * 4 + k
                            out = vps[:, nt_loc // 8, (nt_loc % 8) * D:(nt_loc % 8 + 1) * D]
                            for ct in range(2):
                                nc.tensor.matmul(
                                    out, raw[:, ct, k * 128:(k + 1) * 128], w[:, ct, :],
                                    start=(ct == 0), stop=(ct == 1),
                                )
                    nc.vector.tensor_copy(
                        vaug[:, half * 8:(half + 1) * 8, :, 0:D], vps[:, :, :])
                return vaug

            # ---- attention ----
            def emit_scores_exps(kP, kF, j, g, tag, n_fill):
                """Scores for quad g of chunk j -> two fp8 eblk tiles."""
                ps = [spsum.tile([128, 2, 512], f32, tag="sAB",
                                 name=f"s_{tag}{j}_{g}_{h}")
                      for h in range(2)]
                # HAM filler: dummy matmuls into the quad's own score psum
                # (start=True scores overwrite them; costs no extra banks).
                # Covers the recurring sub-us PE micro-gaps -- without it the
                # clock-gate re-throttles within ~10us of attention start.
                for _ in range(n_fill):
                    nc.tensor.matmul(ps[0][:, 0, 0:256], warm[:, 0:128],
                                     warm[:, 0:256], start=True, stop=True)
                for q in range(4):
                    mt = 4 * g + q
                    nc.tensor.matmul(
                        ps[q // 2][:, q % 2, :],
                        kP[q * 32:(q + 1) * 32, :, mt * 128:(mt + 1) * 128],
                        kF[q * 32:(q + 1) * 32, :, j * 512:(j + 1) * 512],
                        start=True, stop=True,
                        perf_mode=DR, tile_position=(q * 32, 0),
                    )
                ebs = []
                for h in range(2):
                    eblk = ep.tile([128, 2, 512], fp8, tag="eblk",
                                   name=f"e_{tag}{j}_{g}_{h}")
                    idx = exp_idx[0]
                    exp_idx[0] += 1
                    if idx % 2 == 0:
                        nc.scalar.activation(eblk[:, :, :], ps[h][:, :, :],
                                             Exp, scale=SCALE)
                    else:
                        nc.vector.tensor_scalar(
                            eblk[:, :, :].bitcast(i8), ps[h][:, :, :],
                            A_SCH, B_SCH, Alu.mult, Alu.add)
                    ebs.append(eblk)
                return ebs

            def quad(kP, kF, vaug, po, j, g, tag, n_fill=1, pre=None):
                ebs = pre if pre is not None else emit_scores_exps(
                    kP, kF, j, g, tag, n_fill)
                for h in range(2):
                    k = 2 * g + h
                    nc.tensor.matmul(
                        po[:], vaug[:, k, :, :], ebs[h][:, :, :],
                        start=(k == 0), stop=(k == PAIRS - 1),
                        perf_mode=DR,
                    )

            # ---- scheduler fence + HAM warm burst ----
            # Everything DMA-bound stays before the fence; the dependency-free
            # burst after it cannot be hoisted into the idle phase, runs
            # back-to-back, and flips the PE clock-gate to 8/8 right before
            # the attention stream begins.
            tc.no_sync_barrier()

            def warm_burst(n, name):
                wp = spsum.tile([128, 2, 512], f32, tag="sAB", name=name)
                nc.tensor.matmul(
                    wp[:, 0, :], k2f[0:32, :, 4 * 512:4 * 512 + 128],
                    k2f[0:32, :, 4 * 512:5 * 512],
                    start=True, stop=True, perf_mode=DR, tile_position=(0, 0))
                for _ in range(n - 1):
                    nc.tensor.matmul(wp[:, 0, :], warm[:, 0:128], warm[:],
                                     start=True, stop=True)

            warm_burst(26, "warm_att")

            # v1 raw loads after the burst: they stream during early o2 and
            # keep 2MB of DMA out of the pre-attention window (the head is
            # gated by the k-proj matmuls tracking the raw-DMA tail). The
            # projection compute runs at o2 chunk 2, well after they land.
            for u in range(NCH // 2):
                nc.sync.dma_start(
                    out=rawt["v1"][:, :, 2 * u:2 * u + 2, :],
                    in_=v1T[:, (2 * u) * 512:(2 * u + 2) * 512].rearrange(
                        "(c p) (j n) -> p c j n", c=2, j=2),
                )
            v1raws = [rawt["v1"][:, :, j, :] for j in range(NCH)]

            # ---- preheat (emitted before v2aug compute so its score psum
            # tiles are allocated from the pool first and it can run as soon
            # as the early kf staging units land) ----
            preheated = [emit_scores_exps(k2f, k1f, 0, g, "o2", 1)
                         for g in range(3)]

            v2aug = v_proj_compute(v2raws, w_sb["wv2"], "v2aug")

            def run_dir(kP, kF, vaug, oTdr, tag, preheated=(), hook=None):
                for j in range(NCH):
                    po = opsum.tile([80, 512], f32, tag="po", name=f"po_{tag}{j}")
                    for g in range(PAIRS // 2):
                        pre = preheated[g] if j == 0 and g < len(preheated) else None
                        quad(kP, kF, vaug, po, j, g, tag, pre=pre)
                    ot = outp.tile([D + 1, 512], bf16, tag="ot", name=f"ot_{tag}{j}")
                    nc.scalar.copy(ot[:], po[0:D + 1, :])
                    nc.sync.dma_start(out=oTdr[:, j * 512:(j + 1) * 512], in_=ot[:])
                    if hook is not None:
                        hook(j)

            v1aug_h = {}

            def o2_hook(j):
                if j == 2:
                    v1aug_h["t"] = v_proj_compute(v1raws, w_sb["wv1"], "v1aug")

            # o2: E[m, n] = exp(SCALE * k2F[m]*k1F[n]); den over m
            run_dir(k2f, k1f, v2aug, o2Td, "o2", preheated=preheated, hook=o2_hook)
            # o1: E[n, m] = exp(SCALE * k1F[n]*k2F[m]); den over n (o1 scores
            # only need kf, so they fill o2's exp/PV drain -- no gap)
            run_dir(k1f, k2f, v1aug_h["t"], o1Td, "o1")

    nc.compile()
    return nc


def _get_nc():
    if "nc" not in _cache:
        _cache["nc"] = _build_module()
    return _cache["nc"]


def kernel(k1, v1, k2, v2,
           wk1_w, wk1_b, wv1_w, wv1_b,
           wk2_w, wk2_b, wv2_w, wv2_b,
           wo1_w, wo1_b, wo2_w, wo2_b):
    import ml_dtypes
    from concourse.bass_utils import run_bass_kernel_spmd

    nc = _get_nc()

    f = np.float32
    bf = ml_dtypes.bfloat16
    k1T = np.ascontiguousarray(np.asarray(k1, f).T).astype(bf)
    v1T = np.ascontiguousarray(np.asarray(v1, f).T).astype(bf)
    k2T = np.ascontiguousarray(np.asarray(k2, f).T).astype(bf)
    v2T = np.ascontiguousarray(np.asarray(v2, f).T).astype(bf)

    in_maps = []
    for h in range(NCORES):
        sl = slice(h * D, (h + 1) * D)
        in_maps.append({
            "k1T": k1T, "v1T": v1T, "k2T": k2T, "v2T": v2T,
            "wk1": np.ascontiguousarray(np.asarray(wk1_w, f)[:, sl]).astype(bf),
            "wv1": np.ascontiguousarray(np.asarray(wv1_w, f)[:, sl]).astype(bf),
            "wk2": np.ascontiguousarray(np.asarray(wk2_w, f)[:, sl]).astype(bf),
            "wv2": np.ascontiguousarray(np.asarray(wv2_w, f)[:, sl]).astype(bf),
            "bk1": np.ascontiguousarray(np.asarray(wk1_b, f)[sl].reshape(D, 1)),
            "bk2": np.ascontiguousarray(np.asarray(wk2_b, f)[sl].reshape(D, 1)),
        })

    res = run_bass_kernel_spmd(nc, in_maps, list(range(NCORES)))
    _cache["last_result"] = res

    o1_all = np.empty((N, 512), f)
    o2_all = np.empty((N, 512), f)
    for h in range(NCORES):
        rh = res.results[h]
        o1t = np.asarray(rh["o1T"], dtype=f)
        o2t = np.asarray(rh["o2T"], dtype=f)
        o1_all[:, h * D:(h + 1) * D] = (o1t[0:D] / o1t[D:D + 1]).T
        o2_all[:, h * D:(h + 1) * D] = (o2t[0:D] / o2t[D:D + 1]).T
    wo1 = np.asarray(wo1_w, f)
    wo2 = np.asarray(wo2_w, f)
    out1 = o1_all @ wo1 + np.asarray(wv1_b, f) @ wo1 + np.asarray(wo1_b, f)
    out2 = o2_all @ wo2 + np.asarray(wv2_b, f) @ wo2 + np.asarray(wo2_b, f)
    return out1, out2
